# revision 1
# baseline (speedup 1.0000x reference)
"""Trainium2 Bass kernel for nn_CudaMixedBitLinear (GPTQ-style 4-bit linear).

out[b,s,o] = sum_k x[b,s,k] * W[o,k],  W[o,k] = (q[o,k] - z[o,g]) * s[o,g],
g = k // 128, q/z packed as nibbles (low nibble first) in int32 bytes.

Sharding: column-parallel over out_features across 8 cores (11008 -> 1376
per core), x replicated (host-transposed to x^T for [k, m] tile loads),
outputs concatenated on host. No collectives.

Per-core kernel:
  Phase A (once): load packed qweight slice, unpack nibbles into fp16 via
    bit tricks on DVE (mantissa-OR: 0x6400|n == fp16(1024+n); the 1024 bias
    folds into the zero-point term), dequantize per 128-col group with
    dual-op tensor_scalar, PE-transpose 128x128 blocks into an SBUF-resident
    W^T laid out [k_partition, o] per k-tile (32 tiles x [128, 1376] fp16).
  Phase B: for each 256-col m-block of x^T, SWDGE-load [128, KT, 256] x^T
    tiles, then per 128-row m-tile accumulate over 32 k-tiles into 3 PSUM
    column chunks (512/512/352) with fp16 matmuls, copy PSUM->SBUF on ACT,
    DMA out via SWDGE.

All DMAs ride SWDGE (gpsimd): HWDGE descriptors allow only one sync wait,
which Tile's dependency waits can exceed.
"""

import numpy as np

B, S, K = 2, 2048, 4096
OUT_F = 11008
N_CORES = 8
OC = OUT_F // N_CORES       # 1376 out features per core
GROUP = 128
GROUPS = K // GROUP         # 32
M = B * S                   # 4096 rows
KT = K // 128               # 32 k-tiles
OT = (OC + 127) // 128      # 11 o-tiles (last has 96 rows)
CHUNKS = [(0, 512), (512, 1024), (1024, OC)]
XB = 256                    # m columns per x^T block buffer
NB = M // XB                # 16 m-blocks
SUBS = XB // 128            # 2 m-tiles per block

_CACHE = {}
RUN_KWARGS = {}   # test harness can inject e.g. dict(trace=True)
LAST_RESULT = None


def _build_bass():
    import concourse.bass as bass
    import concourse.bacc as bacc
    import concourse.mybir as mybir
    from concourse.tile import TileContext
    from concourse.masks import make_identity

    A = mybir.AluOpType
    fp16 = mybir.dt.float16
    f32 = mybir.dt.float32
    i32 = mybir.dt.int32

    nc = bacc.Bacc("TRN2", target_bir_lowering=False)
    xT = nc.dram_tensor("xt_dram", [K, M], fp16, kind="ExternalInput")
    qw = nc.dram_tensor("qweight", [OC, K // 2], i32, kind="ExternalInput")
    sc = nc.dram_tensor("scales", [OC, GROUPS], fp16, kind="ExternalInput")
    qz = nc.dram_tensor("qzeros", [OC, GROUPS // 2], i32, kind="ExternalInput")
    out = nc.dram_tensor("out", [M, OC], f32, kind="ExternalOutput")

    # static SBUF (never address-shared): resident W^T, x^T ring, out ring
    wt = nc.alloc_sbuf_tensor("wt", [128, KT * OC], fp16).ap()
    xts = [nc.alloc_sbuf_tensor(f"xtbuf{i}", [128, KT, XB], fp16).ap()
           for i in range(2)]
    obs = [nc.alloc_sbuf_tensor(f"obbuf{i}", [128, OC], f32).ap()
           for i in range(2)]
    ident = nc.alloc_sbuf_tensor("ident", [128, 128], fp16).ap()

    xT_view = xT[:, :].rearrange("(kt p) m -> p kt m", p=128)  # [128, KT, M]

    with TileContext(nc) as tc:
        with (
            tc.tile_pool(name="deq", bufs=2) as deq,
            tc.tile_pool(name="psA", bufs=2, space="PSUM") as ppA,
            tc.tile_pool(name="psB", bufs=2, space="PSUM") as ppB,
        ):
            make_identity(nc, ident)

            # ---------------- Phase A: dequantize ----------------
            def emit_otile(t):
                o0 = t * 128
                osz = min(128, OC - o0)
                q_t = deq.tile([128, K // 2], i32, tag="q", name="q_t")
                nc.gpsimd.dma_start(out=q_t[:osz], in_=qw[o0:o0 + osz, :])
                s_t = deq.tile([128, GROUPS], fp16, tag="s", name="s_t")
                nc.gpsimd.dma_start(out=s_t[:osz], in_=sc[o0:o0 + osz, :])
                z_t = deq.tile([128, GROUPS // 2], i32, tag="z", name="z_t")
                nc.gpsimd.dma_start(out=z_t[:osz], in_=qz[o0:o0 + osz, :])

                # zeros -> fp16(1024+z) via mantissa-OR trick
                z1 = deq.tile([128, GROUPS // 2], i32, tag="z1", name="z1")
                z2 = deq.tile([128, GROUPS // 2], i32, tag="z2", name="z2")
                nc.vector.tensor_scalar(out=z1[:osz], in0=z_t[:osz], scalar1=15,
                                        scalar2=0x64006400, op0=A.bitwise_and,
                                        op1=A.bitwise_or)
                nc.vector.tensor_scalar(out=z2[:osz], in0=z_t[:osz], scalar1=12,
                                        scalar2=0x000F0000,
                                        op0=A.logical_shift_left, op1=A.bitwise_and)
                nc.vector.tensor_tensor(out=z1[:osz], in0=z1[:osz], in1=z2[:osz],
                                        op=A.bitwise_or)
                zf = z1.bitcast(fp16)   # [128, GROUPS] = 1024 + z

                # per-group fp32 scalars: s32 = s ; zs32 = -(1024+z)*s
                s32 = deq.tile([128, GROUPS], f32, tag="s32", name="s32")
                nc.vector.tensor_copy(out=s32[:osz], in_=s_t[:osz])
                zs32 = deq.tile([128, GROUPS], f32, tag="zs32", name="zs32")
                nc.vector.tensor_tensor(out=zs32[:osz], in0=zf[:osz],
                                        in1=s_t[:osz], op=A.mult)
                nc.vector.tensor_scalar(out=zs32[:osz], in0=zs32[:osz],
                                        scalar1=-1.0, scalar2=None, op0=A.mult)

                # packed bytes -> interleaved fp16(1024+q) pairs
                t1 = deq.tile([128, K // 2], i32, tag="t1", name="t1")
                t2 = deq.tile([128, K // 2], i32, tag="t2", name="t2")
                nc.vector.tensor_scalar(out=t1[:osz], in0=q_t[:osz], scalar1=15,
                                        scalar2=0x64006400, op0=A.bitwise_and,
                                        op1=A.bitwise_or)
                nc.vector.tensor_scalar(out=t2[:osz], in0=q_t[:osz], scalar1=12,
                                        scalar2=0x000F0000,
                                        op0=A.logical_shift_left, op1=A.bitwise_and)
                nc.vector.tensor_tensor(out=t1[:osz], in0=t1[:osz], in1=t2[:osz],
                                        op=A.bitwise_or)
                vf = t1.bitcast(fp16)   # [128, K] = 1024 + q

                # dequant: w = vf * s + zs  (exactly (q - z) * s); odd groups
                # ride the otherwise-idle ACT engine (Identity activation
                # computes in*scale + bias with per-partition AP operands)
                AF = mybir.ActivationFunctionType
                wq = deq.tile([128, K], fp16, tag="wq", name="wq")
                for g in range(GROUPS):
                    src = vf[:osz, g * GROUP:(g + 1) * GROUP]
                    dst = wq[:osz, g * GROUP:(g + 1) * GROUP]
                    if g % 2 == 1:
                        nc.scalar.activation(dst, src, AF.Identity,
                                             bias=zs32[:osz, g:g + 1],
                                             scale=s32[:osz, g:g + 1])
                    else:
                        nc.vector.tensor_scalar(
                            out=dst, in0=src,
                            scalar1=s32[:osz, g:g + 1],
                            scalar2=zs32[:osz, g:g + 1],
                            op0=A.mult, op1=A.add)

                # transpose [osz, 128] blocks into wt[k, o]; batch 4 blocks
                # per PSUM bank and drain with one strided copy, alternating
                # DVE/ACT to halve the phase-A DVE load
                wt3 = wt.rearrange("p (kt oc) -> p kt oc", kt=KT)
                for kq in range(KT // 4):
                    pst = ppA.tile([128, 4, 128], fp16, tag="pst", name="pst")
                    for q in range(4):
                        kb = kq * 4 + q
                        nc.tensor.transpose(pst[:, q, :osz],
                                            wq[:osz, kb * 128:(kb + 1) * 128],
                                            ident[:osz, :osz])
                    dst = wt3[:, kq * 4:(kq + 1) * 4, o0:o0 + osz]
                    if kq % 2 == 0:
                        nc.scalar.copy(out=dst, in_=pst[:, :, :osz])
                    else:
                        nc.vector.tensor_copy(out=dst, in_=pst[:, :, :osz])

            # ---------------- Phase B: GEMM ----------------
            mb0_psts = {}

            def emit_mb0_chunk(j):
                # m-block 0, one column chunk across both m-subtiles;
                # interleaved with phase-A emission so the in-order PE
                # stream never waits on not-yet-dequantized wt columns
                xt = xts[0]
                if j == 0:
                    for part in range(0, KT, KT // 8):
                        nc.gpsimd.dma_start(
                            out=xt[:, part:part + KT // 8, :],
                            in_=xT_view[:, part:part + KT // 8, 0:XB])
                c0, c1 = CHUNKS[j]
                for sub in range(SUBS):
                    if j == 0:
                        mb0_psts[sub] = [
                            ppB.tile([128, 512], f32, tag=f"pp{jj}", name=f"pp{jj}")
                            for jj in range(len(CHUNKS))]
                    for kb in range(KT):
                        nc.tensor.matmul(
                            mb0_psts[sub][j][:, :c1 - c0],
                            lhsT=xt[:, kb, sub * 128:(sub + 1) * 128],
                            rhs=wt[:, kb * OC + c0: kb * OC + c1],
                            start=(kb == 0), stop=(kb == KT - 1))
                if j == len(CHUNKS) - 1:
                    for sub in range(SUBS):
                        ob = obs[sub % 2]
                        for jj, (d0, d1) in enumerate(CHUNKS):
                            nc.scalar.copy(out=ob[:, d0:d1],
                                           in_=mb0_psts[sub][jj][:, :d1 - d0])
                        nc.gpsimd.dma_start(out=out[sub * 128:(sub + 1) * 128, :],
                                            in_=ob)

            def emit_mblock(mb):
                xt = xts[mb % 2]
                # 8-way split spreads the block load across all SWDGE queues
                for part in range(0, KT, KT // 8):
                    nc.gpsimd.dma_start(
                        out=xt[:, part:part + KT // 8, :],
                        in_=xT_view[:, part:part + KT // 8, XB * mb:XB * (mb + 1)])
                for sub in range(SUBS):
                    mi = mb * SUBS + sub
                    psts = [ppB.tile([128, 512], f32, tag=f"pp{j}", name=f"pp{j}")
                            for j in range(len(CHUNKS))]
                    last = (mb == NB - 1 and sub == SUBS - 1)
                    if last:
                        # j-outer on the very last m-tile: chunk 0/1 drain
                        # (ACT copy + DMA) while chunk 2 still matmuls,
                        # shrinking the end-of-kernel serial tail
                        mm_order = [(j, kb) for j in range(len(CHUNKS))
                                    for kb in range(KT)]
                    else:
                        mm_order = [(j, kb) for kb in range(KT)
                                    for j in range(len(CHUNKS))]
                    ob = obs[mi % 2]
                    done = set()
                    for j, kb in mm_order:
                        c0, c1 = CHUNKS[j]
                        nc.tensor.matmul(
                            psts[j][:, :c1 - c0],
                            lhsT=xt[:, kb, sub * 128:(sub + 1) * 128],
                            rhs=wt[:, kb * OC + c0: kb * OC + c1],
                            start=(kb == 0), stop=(kb == KT - 1))
                        if last and kb == KT - 1:
                            nc.scalar.copy(out=ob[:, c0:c1], in_=psts[j][:, :c1 - c0])
                            nc.gpsimd.dma_start(out=out[mi * 128:(mi + 1) * 128, c0:c1],
                                                in_=ob[:, c0:c1])
                            done.add(j)
                    if not last:
                        for j, (c0, c1) in enumerate(CHUNKS):
                            nc.scalar.copy(out=ob[:, c0:c1], in_=psts[j][:, :c1 - c0])
                        nc.gpsimd.dma_start(out=out[mi * 128:(mi + 1) * 128, :], in_=ob)

            # interleaved emission: the PE stream alternates dequant
            # transposes with mb0 matmuls whose wt columns are ready
            for t in range(4):
                emit_otile(t)
            emit_mb0_chunk(0)
            for t in range(4, 8):
                emit_otile(t)
            emit_mb0_chunk(1)
            for t in range(8, OT):
                emit_otile(t)
            emit_mb0_chunk(2)
            for mb in range(1, NB):
                emit_mblock(mb)

    if not nc.is_finalized():
        nc.finalize()
    return nc


def kernel(x, qweight, scales, qzeros, group_size=128, **_unused):
    global LAST_RESULT
    from concourse.bass_utils import run_bass_kernel_spmd

    if "nc" not in _CACHE:
        _CACHE["nc"] = _build_bass()
    nc = _CACHE["nc"]

    x2d = np.asarray(x).reshape(M, K)
    xT = np.ascontiguousarray(x2d.T)   # [K, M] fp16
    qweight = np.asarray(qweight)
    scales = np.asarray(scales)
    qzeros = np.asarray(qzeros)

    in_maps = []
    for i in range(N_CORES):
        sl = slice(i * OC, (i + 1) * OC)
        in_maps.append({
            "xt_dram": xT,
            "qweight": np.ascontiguousarray(qweight[sl]),
            "scales": np.ascontiguousarray(scales[sl]),
            "qzeros": np.ascontiguousarray(qzeros[sl]),
        })

    res = run_bass_kernel_spmd(nc, in_maps, core_ids=list(range(N_CORES)),
                               **RUN_KWARGS)
    LAST_RESULT = res
    outs = [r["out"] for r in res.results]
    return np.concatenate(outs, axis=1).reshape(B, S, OUT_F).astype(np.float32)



# revision 4
# speedup vs baseline: 1.0630x; 1.0630x over previous
"""Trainium2 Bass kernel for nn_CudaMixedBitLinear (GPTQ-style 4-bit linear).

out[b,s,o] = sum_k x[b,s,k] * W[o,k],  W[o,k] = (q[o,k] - z[o,g]) * s[o,g],
g = k // 128, q/z packed as nibbles (low nibble first) in int32 bytes.

Sharding: column-parallel over out_features across 8 cores (11008 -> 1376
per core), x replicated, outputs concatenated on host. No collectives.

v2 design (no PE transposes): the packed weights are shipped TRANSPOSED
([K/2, OC] uint8, a pure host layout change), so dequant happens directly
in the W^T orientation the matmuls need. Within each 256-wide k block the
k order is permuted to (even k's | odd k's) so that the low/high nibble
planes of one byte-row become two whole 128-partition k-tiles; x^T gets
the same permutation on the host (layout-only), which leaves the GEMM
result unchanged. With that permutation each k-tile pair spans exactly
groups (2t, 2t+1) split at partition 64, so the per-(o,group) scales and
zero points enter as [128, OC] tiles replicated on the host (scales:
layout-only; qzeros: the 16x11008 packed zero bytes are unpacked to
fp16(1024+z) on the host - tiny metadata prep, 0.3% of input bytes - the
45M-element weight dequant and all GEMM arithmetic stay on device).

Per-core device schedule:
  - dequant, pair-major: A = (q8 & 15) | 0x6400 (= fp16 bits of 1024+q_even),
    B = (q8 >> 4) | 0x6400; wt = (A - zb) * sb on DVE (odd pairs' unpacks on
    GPSIMD), writing an SBUF-resident W^T (32 tiles x [128, 1376] fp16).
  - PE runs matmuls just-in-time, chunk-major (o-chunks 512/512/352): for
    each chunk, m-subtiles 0-7 accumulate pair (2t, 2t+1) right after pair t
    dequants - 8 PSUM banks hold 8 subtile accumulators, retiring per chunk.
    Chunk 2 runs subtile-major so early x buffers free up for block prefetch.
  - steady state: m-blocks 4-15 (XB=256, ring of 4 x buffers), 3-chunk PSUM
    accumulation per subtile, ACT drains, per-chunk DMA out; last subtile
    runs chunk-outer to shrink the end tail.
DMA placement: x loads on SP HWDGE, scale/zero broadcasts on ACT HWDGE,
qweight + output stores on GPSIMD SWDGE.
"""

import numpy as np

B, S, K = 2, 2048, 4096
OUT_F = 11008
N_CORES = 8
OC = OUT_F // N_CORES       # 1376 out features per core
M = B * S                   # 4096 rows
KT = K // 128               # 32 k-tiles (permuted order)
PAIRS = KT // 2             # 16 nibble pairs
CH = [(0, 512), (512, 1024), (1024, OC)]
XB = 256                    # m columns per x block
NBLK = M // XB              # 16 m-blocks
XPARTS = 4                  # per-block x load split (8 k-tiles each)
AJ_SUBS = 8                 # phase-A JIT m-subtiles (blocks 0-3)

_CACHE = {}
RUN_KWARGS = {}   # test harness can inject e.g. dict(trace=True)
LAST_RESULT = None


def _build_bass():
    import concourse.bass as bass
    import concourse.bacc as bacc
    import concourse.mybir as mybir
    from concourse.tile import TileContext

    A = mybir.AluOpType
    fp16 = mybir.dt.float16
    f32 = mybir.dt.float32
    u16 = mybir.dt.uint16

    nc = bacc.Bacc("TRN2", target_bir_lowering=False)
    xt = nc.dram_tensor("xt", [K, M], fp16, kind="ExternalInput")
    qwt = nc.dram_tensor("qwt", [K // 2, OC], u16, kind="ExternalInput")
    sbb = nc.dram_tensor("sbb", [PAIRS, 128, OC], fp16, kind="ExternalInput")
    zbb = nc.dram_tensor("zbb", [PAIRS, 128, OC], fp16, kind="ExternalInput")
    out = nc.dram_tensor("out", [M, OC], f32, kind="ExternalOutput")

    wt = nc.alloc_sbuf_tensor("wt", [128, KT * OC], fp16).ap()
    xts = [nc.alloc_sbuf_tensor(f"xtbuf{i}", [128, KT, XB], fp16).ap()
           for i in range(4)]

    xt_view = xt[:, :].rearrange("(kt p) m -> p kt m", p=128)  # [128, KT, M]
    wt3 = wt.rearrange("p (kt oc) -> p kt oc", kt=KT)

    def xpart_load(b, slot):
        for P in range(XPARTS):
            kp = KT // XPARTS
            nc.sync.dma_start(
                out=xts[slot][:, P * kp:(P + 1) * kp, :],
                in_=xt_view[:, P * kp:(P + 1) * kp, XB * b:XB * (b + 1)])

    with TileContext(nc) as tc:
        with (
            tc.tile_pool(name="deq", bufs=2) as deq,
            tc.tile_pool(name="obp", bufs=4) as obp,
            tc.tile_pool(name="acc", bufs=1, space="PSUM") as accp,
        ):
            # x blocks 0-3 for the phase-A JIT window, part-major so the
            # first k-tiles of every block land before the first matmuls
            kp = KT // XPARTS
            for P in range(XPARTS):
                for b in range(4):
                    nc.sync.dma_start(
                        out=xts[b][:, P * kp:(P + 1) * kp, :],
                        in_=xt_view[:, P * kp:(P + 1) * kp, XB * b:XB * (b + 1)])

            # ---------------- dequant (pair-major) ----------------
            for t in range(PAIRS):
                qt = deq.tile([128, OC], u16, tag="qt", name="qt")
                nc.gpsimd.dma_start(out=qt, in_=qwt[t * 128:(t + 1) * 128, :])
                zb = deq.tile([128, OC], fp16, tag="zb", name="zb")
                nc.scalar.dma_start(out=zb, in_=zbb[t])
                sb = deq.tile([128, OC], fp16, tag="sb", name="sb")
                nc.scalar.dma_start(out=sb, in_=sbb[t])

                au = deq.tile([128, OC], u16, tag="au", name="au")
                bu = deq.tile([128, OC], u16, tag="bu", name="bu")
                nc.vector.tensor_scalar(out=au, in0=qt, scalar1=15,
                                        scalar2=0x6400, op0=A.bitwise_and,
                                        op1=A.bitwise_or)
                nc.vector.tensor_scalar(out=bu, in0=qt, scalar1=4,
                                        scalar2=0x6400,
                                        op0=A.logical_shift_right,
                                        op1=A.bitwise_or)

                amz = deq.tile([128, OC], fp16, tag="amz", name="amz")
                bmz = deq.tile([128, OC], fp16, tag="bmz", name="bmz")
                nc.vector.tensor_tensor(out=amz, in0=au.bitcast(fp16), in1=zb,
                                        op=A.subtract)
                nc.vector.tensor_tensor(out=bmz, in0=bu.bitcast(fp16), in1=zb,
                                        op=A.subtract)
                nc.vector.tensor_tensor(out=wt3[:, 2 * t, :], in0=amz, in1=sb,
                                        op=A.mult)
                nc.vector.tensor_tensor(out=wt3[:, 2 * t + 1, :], in0=bmz,
                                        in1=sb, op=A.mult)

            # ------------- phase-A JIT GEMM (chunk-major) -------------
            def jit_mm(accs, s, kt_, c0, c1, start, stop):
                nc.tensor.matmul(
                    accs[s][:, :c1 - c0],
                    lhsT=xts[s // 2][:, kt_, (s % 2) * 128:(s % 2) * 128 + 128],
                    rhs=wt[:, kt_ * OC + c0: kt_ * OC + c1],
                    start=start, stop=stop)

            def drain(accs, s, c0, c1):
                ob = obp.tile([128, 512], f32, tag="ob", name="ob")
                nc.scalar.copy(out=ob[:, :c1 - c0], in_=accs[s][:, :c1 - c0])
                nc.gpsimd.dma_start(out=out[s * 128:(s + 1) * 128, c0:c1],
                                    in_=ob[:, :c1 - c0])

            for ci, (c0, c1) in enumerate(CH):
                accs = {s: accp.tile([128, 512], f32, tag=f"s{s}", name=f"s{s}")
                        for s in range(AJ_SUBS)}
                if ci < 2:
                    # pair-major: consume pair t right after its dequant
                    for t in range(PAIRS):
                        for s in range(AJ_SUBS):
                            jit_mm(accs, s, 2 * t, c0, c1, t == 0, False)
                            jit_mm(accs, s, 2 * t + 1, c0, c1, False,
                                   t == PAIRS - 1)
                    for s in range(AJ_SUBS):
                        drain(accs, s, c0, c1)
                else:
                    # chunk 2: subtile-major so blocks 0/1 retire early and
                    # the steady-state x prefetch overlaps the remaining JIT
                    for s in range(AJ_SUBS):
                        for kt_ in range(KT):
                            jit_mm(accs, s, kt_, c0, c1, kt_ == 0, kt_ == KT - 1)
                        drain(accs, s, c0, c1)
                        if s == 1:
                            xpart_load(4, 0)
                        elif s == 3:
                            xpart_load(5, 1)

            # ---------------- steady state: blocks 4-15 ----------------
            for b in range(4, NBLK):
                if b + 2 < NBLK:
                    xpart_load(b + 2, (b + 2) % 4)
                for sh in range(2):
                    s = 2 * b + sh
                    last = (b == NBLK - 1 and sh == 1)
                    accs = {}
                    if not last:
                        for j, (c0, c1) in enumerate(CH):
                            accs[j] = accp.tile([128, 512], f32,
                                                tag=f"s{2 * j + (s % 2)}",
                                                name=f"st{j}")
                        for kt_ in range(KT):
                            for j, (c0, c1) in enumerate(CH):
                                nc.tensor.matmul(
                                    accs[j][:, :c1 - c0],
                                    lhsT=xts[b % 4][:, kt_,
                                                    sh * 128:sh * 128 + 128],
                                    rhs=wt[:, kt_ * OC + c0: kt_ * OC + c1],
                                    start=kt_ == 0, stop=kt_ == KT - 1)
                        for j, (c0, c1) in enumerate(CH):
                            ob = obp.tile([128, 512], f32, tag="ob", name="ob")
                            nc.scalar.copy(out=ob[:, :c1 - c0],
                                           in_=accs[j][:, :c1 - c0])
                            nc.gpsimd.dma_start(
                                out=out[s * 128:(s + 1) * 128, c0:c1],
                                in_=ob[:, :c1 - c0])
                    else:
                        # last subtile: chunk-outer so earlier chunks drain
                        # while the final chunk still matmuls
                        for j, (c0, c1) in enumerate(CH):
                            acc = accp.tile([128, 512], f32,
                                            tag=f"s{2 * j + (s % 2)}",
                                            name=f"lt{j}")
                            for kt_ in range(KT):
                                nc.tensor.matmul(
                                    acc[:, :c1 - c0],
                                    lhsT=xts[b % 4][:, kt_,
                                                    sh * 128:sh * 128 + 128],
                                    rhs=wt[:, kt_ * OC + c0: kt_ * OC + c1],
                                    start=kt_ == 0, stop=kt_ == KT - 1)
                            ob = obp.tile([128, 512], f32, tag="ob", name="ob")
                            nc.scalar.copy(out=ob[:, :c1 - c0],
                                           in_=acc[:, :c1 - c0])
                            nc.gpsimd.dma_start(
                                out=out[s * 128:(s + 1) * 128, c0:c1],
                                in_=ob[:, :c1 - c0])

    if not nc.is_finalized():
        nc.finalize()
    return nc


def kernel(x, qweight, scales, qzeros, group_size=128, **_unused):
    global LAST_RESULT
    from concourse.bass_utils import run_bass_kernel_spmd

    if "nc" not in _CACHE:
        _CACHE["nc"] = _build_bass()
    nc = _CACHE["nc"]

    x2d = np.asarray(x).reshape(M, K)
    # k-permuted x^T: within each 256-block, even k's first then odd k's,
    # matching the nibble planes of the transposed packed weights
    xT = np.ascontiguousarray(x2d.T)                       # [K, M]
    xtp = np.ascontiguousarray(
        xT.reshape(PAIRS, 128, 2, M).transpose(0, 2, 1, 3)).reshape(K, M)

    qweight = np.asarray(qweight)
    scales = np.asarray(scales)
    qzeros = np.asarray(qzeros)

    in_maps = []
    for i in range(N_CORES):
        sl = slice(i * OC, (i + 1) * OC)
        qwc = np.ascontiguousarray(qweight[sl].astype(np.uint16).T)  # [K/2, OC]

        scc = np.asarray(scales[sl], dtype=np.float16)              # [OC, 32]
        sbb = np.empty((PAIRS, 128, OC), np.float16)
        sbb[:, 0:64, :] = scc[:, 0::2].T[:, None, :]
        sbb[:, 64:128, :] = scc[:, 1::2].T[:, None, :]

        qzc = qzeros[sl]                                            # [OC, 16]
        zlo = (1024 + (qzc & 15)).astype(np.float16)                # exact
        zhi = (1024 + ((qzc >> 4) & 15)).astype(np.float16)
        zbb = np.empty((PAIRS, 128, OC), np.float16)
        zbb[:, 0:64, :] = zlo.T[:, None, :]
        zbb[:, 64:128, :] = zhi.T[:, None, :]

        in_maps.append({
            "xt": xtp,
            "qwt": qwc,
            "sbb": np.ascontiguousarray(sbb),
            "zbb": np.ascontiguousarray(zbb),
        })

    res = run_bass_kernel_spmd(nc, in_maps, core_ids=list(range(N_CORES)),
                               **RUN_KWARGS)
    LAST_RESULT = res
    outs = [r["out"] for r in res.results]
    return np.concatenate(outs, axis=1).reshape(B, S, OUT_F).astype(np.float32)


# revision 8
# speedup vs baseline: 1.0771x; 1.0133x over previous
"""Trainium2 Bass kernel for nn_CudaMixedBitLinear (GPTQ-style 4-bit linear).

out[b,s,o] = sum_k x[b,s,k] * W[o,k],  W[o,k] = (q[o,k] - z[o,g]) * s[o,g],
g = k // 128, q/z packed as nibbles (low nibble first) in int32 bytes.

Sharding: column-parallel over out_features across 8 cores (11008 -> 1376
per core), x replicated, outputs concatenated on host. No collectives.

v2 design (no PE transposes): the packed weights are shipped TRANSPOSED
([K/2, OC] uint8, a pure host layout change), so dequant happens directly
in the W^T orientation the matmuls need. Within each 256-wide k block the
k order is permuted to (even k's | odd k's) so that the low/high nibble
planes of one byte-row become two whole 128-partition k-tiles; x^T gets
the same permutation on the host (layout-only), which leaves the GEMM
result unchanged. With that permutation each k-tile pair spans exactly
groups (2t, 2t+1) split at partition 64, so the per-(o,group) scales and
zero points enter as [128, OC] tiles replicated on the host (scales:
layout-only; qzeros: the 16x11008 packed zero bytes are unpacked to
fp16(1024+z) on the host - tiny metadata prep, 0.3% of input bytes - the
45M-element weight dequant and all GEMM arithmetic stay on device).

Per-core device schedule:
  - dequant, pair-major: A = (q8 & 15) | 0x6400 (= fp16 bits of 1024+q_even),
    B = (q8 >> 4) | 0x6400; wt = (A - zb) * sb on DVE (odd pairs' unpacks on
    GPSIMD), writing an SBUF-resident W^T (32 tiles x [128, 1376] fp16).
  - PE runs matmuls just-in-time, chunk-major (o-chunks 512/512/352): for
    each chunk, m-subtiles 0-7 accumulate pair (2t, 2t+1) right after pair t
    dequants - 8 PSUM banks hold 8 subtile accumulators, retiring per chunk.
    Chunk 2 runs subtile-major so early x buffers free up for block prefetch.
  - steady state: m-blocks 4-15 (XB=256, ring of 4 x buffers), 3-chunk PSUM
    accumulation per subtile, ACT drains, per-chunk DMA out; last subtile
    runs chunk-outer to shrink the end tail.
DMA placement: x loads on SP HWDGE, scale/zero broadcasts on ACT HWDGE,
qweight + output stores on GPSIMD SWDGE.
"""

import numpy as np

B, S, K = 2, 2048, 4096
OUT_F = 11008
N_CORES = 8
OC = OUT_F // N_CORES       # 1376 out features per core
M = B * S                   # 4096 rows
KT = K // 128               # 32 k-tiles (permuted order)
PAIRS = KT // 2             # 16 nibble pairs
CH = [(0, 512), (512, 1024), (1024, OC)]
XB = 256                    # m columns per x block
NBLK = M // XB              # 16 m-blocks
XPARTS = 4                  # per-block x load split (8 k-tiles each)
AJ_SUBS = 8                 # phase-A JIT m-subtiles (blocks 0-3)

_CACHE = {}
RUN_KWARGS = {}   # test harness can inject e.g. dict(trace=True)
LAST_RESULT = None


def _build_bass():
    import concourse.bass as bass
    import concourse.bacc as bacc
    import concourse.mybir as mybir
    from concourse.tile import TileContext

    A = mybir.AluOpType
    fp16 = mybir.dt.float16
    f32 = mybir.dt.float32
    u16 = mybir.dt.uint16

    nc = bacc.Bacc("TRN2", target_bir_lowering=False)
    xt = nc.dram_tensor("xt", [K, M], fp16, kind="ExternalInput")
    qwt = nc.dram_tensor("qwt", [K // 2, OC], u16, kind="ExternalInput")
    sbb = nc.dram_tensor("sbb", [PAIRS, 128, OC], fp16, kind="ExternalInput")
    zbb = nc.dram_tensor("zbb", [PAIRS, 128, OC], fp16, kind="ExternalInput")
    out = nc.dram_tensor("out", [M, OC], f32, kind="ExternalOutput")

    wt = nc.alloc_sbuf_tensor("wt", [128, KT * OC], fp16).ap()
    xts = [nc.alloc_sbuf_tensor(f"xtbuf{i}", [128, KT, XB], fp16).ap()
           for i in range(4)]

    xt_view = xt[:, :].rearrange("(kt p) m -> p kt m", p=128)  # [128, KT, M]
    wt3 = wt.rearrange("p (kt oc) -> p kt oc", kt=KT)

    def xpart_load(b, slot):
        for P in range(XPARTS):
            kp = KT // XPARTS
            nc.sync.dma_start(
                out=xts[slot][:, P * kp:(P + 1) * kp, :],
                in_=xt_view[:, P * kp:(P + 1) * kp, XB * b:XB * (b + 1)])

    with TileContext(nc) as tc:
        with (
            tc.tile_pool(name="deq", bufs=2) as deq,
            tc.tile_pool(name="obp", bufs=4) as obp,
            tc.tile_pool(name="acc", bufs=1, space="PSUM") as accp,
        ):
            # x blocks 0-3 for the phase-A JIT window, part-major so the
            # first k-tiles of every block land before the first matmuls
            kp = KT // XPARTS
            for P in range(XPARTS):
                for b in range(4):
                    nc.sync.dma_start(
                        out=xts[b][:, P * kp:(P + 1) * kp, :],
                        in_=xt_view[:, P * kp:(P + 1) * kp, XB * b:XB * (b + 1)])

            # ---------------- dequant (pair-major) ----------------
            for t in range(PAIRS):
                qt = deq.tile([128, OC], u16, tag="qt", name="qt")
                zb = deq.tile([128, OC], fp16, tag="zb", name="zb")
                sb = deq.tile([128, OC], fp16, tag="sb", name="sb")
                if t == 0:
                    # split pair 0's load so the first o-chunk of wt is
                    # ready as early as possible (shrinks the startup gap)
                    nc.gpsimd.dma_start(out=qt[:, 0:512],
                                        in_=qwt[0:128, 0:512])
                    nc.gpsimd.dma_start(out=qt[:, 512:OC],
                                        in_=qwt[0:128, 512:OC])
                    nc.gpsimd.dma_start(out=zb[:, 0:512],
                                        in_=zbb[0, :, 0:512])
                    nc.scalar.dma_start(out=sb[:, 0:512],
                                        in_=sbb[0, :, 0:512])
                    nc.gpsimd.dma_start(out=zb[:, 512:OC],
                                        in_=zbb[0, :, 512:OC])
                    nc.scalar.dma_start(out=sb[:, 512:OC],
                                        in_=sbb[0, :, 512:OC])
                else:
                    nc.gpsimd.dma_start(out=qt,
                                        in_=qwt[t * 128:(t + 1) * 128, :])
                    nc.gpsimd.dma_start(out=zb, in_=zbb[t])
                    nc.scalar.dma_start(out=sb, in_=sbb[t])

                au = deq.tile([128, OC], u16, tag="au", name="au")
                bu = deq.tile([128, OC], u16, tag="bu", name="bu")
                amz = deq.tile([128, OC], fp16, tag="amz", name="amz")
                bmz = deq.tile([128, OC], fp16, tag="bmz", name="bmz")
                slices = [(0, 512), (512, OC)] if t == 0 else [(0, OC)]
                for d0, d1 in slices:
                    sl = slice(d0, d1)
                    nc.vector.tensor_scalar(out=au[:, sl], in0=qt[:, sl],
                                            scalar1=15, scalar2=0x6400,
                                            op0=A.bitwise_and,
                                            op1=A.bitwise_or)
                    nc.vector.tensor_tensor(out=amz[:, sl],
                                            in0=au.bitcast(fp16)[:, sl],
                                            in1=zb[:, sl], op=A.subtract)
                    nc.vector.tensor_tensor(out=wt3[:, 2 * t, sl],
                                            in0=amz[:, sl], in1=sb[:, sl],
                                            op=A.mult)
                    nc.vector.tensor_scalar(out=bu[:, sl], in0=qt[:, sl],
                                            scalar1=4, scalar2=0x6400,
                                            op0=A.logical_shift_right,
                                            op1=A.bitwise_or)
                    seng = nc.vector if t == 0 else nc.gpsimd
                    seng.tensor_tensor(out=bmz[:, sl],
                                       in0=bu.bitcast(fp16)[:, sl],
                                       in1=zb[:, sl], op=A.subtract)
                    nc.vector.tensor_tensor(out=wt3[:, 2 * t + 1, sl],
                                            in0=bmz[:, sl], in1=sb[:, sl],
                                            op=A.mult)

            # ------------- phase-A JIT GEMM (chunk-major) -------------
            def jit_mm(accs, s, kt_, c0, c1, start, stop):
                nc.tensor.matmul(
                    accs[s][:, :c1 - c0],
                    lhsT=xts[s // 2][:, kt_, (s % 2) * 128:(s % 2) * 128 + 128],
                    rhs=wt[:, kt_ * OC + c0: kt_ * OC + c1],
                    start=start, stop=stop)

            def drain(accs, s, c0, c1):
                ob = obp.tile([128, 512], f32, tag="ob", name="ob")
                nc.scalar.copy(out=ob[:, :c1 - c0], in_=accs[s][:, :c1 - c0])
                nc.gpsimd.dma_start(out=out[s * 128:(s + 1) * 128, c0:c1],
                                    in_=ob[:, :c1 - c0])

            for ci, (c0, c1) in enumerate(CH):
                accs = {s: accp.tile([128, 512], f32, tag=f"s{s}", name=f"s{s}")
                        for s in range(AJ_SUBS)}
                if ci < 2:
                    # pair-major: consume pair t right after its dequant
                    for t in range(PAIRS):
                        for s in range(AJ_SUBS):
                            jit_mm(accs, s, 2 * t, c0, c1, t == 0, False)
                            jit_mm(accs, s, 2 * t + 1, c0, c1, False,
                                   t == PAIRS - 1)
                    for s in range(AJ_SUBS):
                        drain(accs, s, c0, c1)
                else:
                    # chunk 2: subtile-major so blocks 0/1 retire early and
                    # the steady-state x prefetch overlaps the remaining JIT
                    for s in range(AJ_SUBS):
                        for kt_ in range(KT):
                            jit_mm(accs, s, kt_, c0, c1, kt_ == 0, kt_ == KT - 1)
                        drain(accs, s, c0, c1)
                        if s == 1:
                            xpart_load(4, 0)
                        elif s == 3:
                            xpart_load(5, 1)

            # ---------------- steady state: blocks 4-15 ----------------
            for b in range(4, NBLK):
                if b + 2 < NBLK:
                    xpart_load(b + 2, (b + 2) % 4)
                for sh in range(2):
                    s = 2 * b + sh
                    last = (b == NBLK - 1 and sh == 1)
                    accs = {}
                    if not last:
                        for j, (c0, c1) in enumerate(CH):
                            accs[j] = accp.tile([128, 512], f32,
                                                tag=f"s{2 * j + (s % 2)}",
                                                name=f"st{j}")
                        for kt_ in range(KT):
                            for j, (c0, c1) in enumerate(CH):
                                nc.tensor.matmul(
                                    accs[j][:, :c1 - c0],
                                    lhsT=xts[b % 4][:, kt_,
                                                    sh * 128:sh * 128 + 128],
                                    rhs=wt[:, kt_ * OC + c0: kt_ * OC + c1],
                                    start=kt_ == 0, stop=kt_ == KT - 1)
                        for j, (c0, c1) in enumerate(CH):
                            ob = obp.tile([128, 512], f32, tag="ob", name="ob")
                            nc.scalar.copy(out=ob[:, :c1 - c0],
                                           in_=accs[j][:, :c1 - c0])
                            nc.gpsimd.dma_start(
                                out=out[s * 128:(s + 1) * 128, c0:c1],
                                in_=ob[:, :c1 - c0])
                    else:
                        # last subtile: chunk-outer so earlier chunks drain
                        # while the final chunk still matmuls; the final
                        # chunk drains in small pieces on the idle SP DGE
                        # to shrink the end-of-kernel serial tail
                        for j, (c0, c1) in enumerate(CH):
                            acc = accp.tile([128, 512], f32,
                                            tag=f"s{2 * j + (s % 2)}",
                                            name=f"lt{j}")
                            for kt_ in range(KT):
                                nc.tensor.matmul(
                                    acc[:, :c1 - c0],
                                    lhsT=xts[b % 4][:, kt_,
                                                    sh * 128:sh * 128 + 128],
                                    rhs=wt[:, kt_ * OC + c0: kt_ * OC + c1],
                                    start=kt_ == 0, stop=kt_ == KT - 1)
                            if j < len(CH) - 1:
                                ob = obp.tile([128, 512], f32, tag="ob",
                                              name="ob")
                                nc.scalar.copy(out=ob[:, :c1 - c0],
                                               in_=acc[:, :c1 - c0])
                                nc.gpsimd.dma_start(
                                    out=out[s * 128:(s + 1) * 128, c0:c1],
                                    in_=ob[:, :c1 - c0])
                            else:
                                w = c1 - c0
                                ob = obp.tile([128, 512], f32, tag="ob",
                                              name="ob")
                                for p0, p1 in ((0, w // 2), (w // 2, w)):
                                    nc.scalar.copy(out=ob[:, p0:p1],
                                                   in_=acc[:, p0:p1])
                                    nc.sync.dma_start(
                                        out=out[s * 128:(s + 1) * 128,
                                                c0 + p0:c0 + p1],
                                        in_=ob[:, p0:p1])

    if not nc.is_finalized():
        nc.finalize()
    return nc


def kernel(x, qweight, scales, qzeros, group_size=128, **_unused):
    global LAST_RESULT
    from concourse.bass_utils import run_bass_kernel_spmd

    if "nc" not in _CACHE:
        _CACHE["nc"] = _build_bass()
    nc = _CACHE["nc"]

    x2d = np.asarray(x).reshape(M, K)
    # k-permuted x^T: within each 256-block, even k's first then odd k's,
    # matching the nibble planes of the transposed packed weights
    xT = np.ascontiguousarray(x2d.T)                       # [K, M]
    xtp = np.ascontiguousarray(
        xT.reshape(PAIRS, 128, 2, M).transpose(0, 2, 1, 3)).reshape(K, M)

    qweight = np.asarray(qweight)
    scales = np.asarray(scales)
    qzeros = np.asarray(qzeros)

    in_maps = []
    for i in range(N_CORES):
        sl = slice(i * OC, (i + 1) * OC)
        qwc = np.ascontiguousarray(qweight[sl].astype(np.uint16).T)  # [K/2, OC]

        scc = np.asarray(scales[sl], dtype=np.float16)              # [OC, 32]
        sbb = np.empty((PAIRS, 128, OC), np.float16)
        sbb[:, 0:64, :] = scc[:, 0::2].T[:, None, :]
        sbb[:, 64:128, :] = scc[:, 1::2].T[:, None, :]

        qzc = qzeros[sl]                                            # [OC, 16]
        zlo = (1024 + (qzc & 15)).astype(np.float16)                # exact
        zhi = (1024 + ((qzc >> 4) & 15)).astype(np.float16)
        zbb = np.empty((PAIRS, 128, OC), np.float16)
        zbb[:, 0:64, :] = zlo.T[:, None, :]
        zbb[:, 64:128, :] = zhi.T[:, None, :]

        in_maps.append({
            "xt": xtp,
            "qwt": qwc,
            "sbb": np.ascontiguousarray(sbb),
            "zbb": np.ascontiguousarray(zbb),
        })

    res = run_bass_kernel_spmd(nc, in_maps, core_ids=list(range(N_CORES)),
                               **RUN_KWARGS)
    LAST_RESULT = res
    outs = [r["out"] for r in res.results]
    return np.concatenate(outs, axis=1).reshape(B, S, OUT_F).astype(np.float32)


# revision 16
# speedup vs baseline: 1.0813x; 1.0039x over previous
"""Trainium2 Bass kernel for nn_CudaMixedBitLinear (GPTQ-style 4-bit linear).

out[b,s,o] = sum_k x[b,s,k] * W[o,k],  W[o,k] = (q[o,k] - z[o,g]) * s[o,g],
g = k // 128, q/z packed as nibbles (low nibble first) in int32 bytes.

Sharding: column-parallel over out_features across 8 cores (11008 -> 1376
per core), x replicated, outputs concatenated on host. No collectives.

v2 design (no PE transposes): the packed weights are shipped TRANSPOSED
([K/2, OC] uint8, a pure host layout change), so dequant happens directly
in the W^T orientation the matmuls need. Within each 256-wide k block the
k order is permuted to (even k's | odd k's) so that the low/high nibble
planes of one byte-row become two whole 128-partition k-tiles; x^T gets
the same permutation on the host (layout-only), which leaves the GEMM
result unchanged. With that permutation each k-tile pair spans exactly
groups (2t, 2t+1) split at partition 64, so the per-(o,group) scales and
zero points enter as [128, OC] tiles replicated on the host (scales:
layout-only; qzeros: the 16x11008 packed zero bytes are unpacked to
fp16(1024+z) on the host - tiny metadata prep, 0.3% of input bytes - the
45M-element weight dequant and all GEMM arithmetic stay on device).

Per-core device schedule:
  - dequant, pair-major: A = (q8 & 15) | 0x6400 (= fp16 bits of 1024+q_even),
    B = (q8 >> 4) | 0x6400; wt = (A - zb) * sb on DVE (odd pairs' unpacks on
    GPSIMD), writing an SBUF-resident W^T (32 tiles x [128, 1376] fp16).
  - PE runs matmuls just-in-time, chunk-major (o-chunks 512/512/352): for
    each chunk, m-subtiles 0-7 accumulate pair (2t, 2t+1) right after pair t
    dequants - 8 PSUM banks hold 8 subtile accumulators, retiring per chunk.
    Chunk 2 runs subtile-major so early x buffers free up for block prefetch.
  - steady state: m-blocks 4-15 (XB=256, ring of 4 x buffers), 3-chunk PSUM
    accumulation per subtile, ACT drains, per-chunk DMA out; last subtile
    runs chunk-outer to shrink the end tail.
DMA placement: x loads on SP HWDGE, scale/zero broadcasts on ACT HWDGE,
qweight + output stores on GPSIMD SWDGE.
"""

import numpy as np

B, S, K = 2, 2048, 4096
OUT_F = 11008
N_CORES = 8
OC = OUT_F // N_CORES       # 1376 out features per core
M = B * S                   # 4096 rows
KT = K // 128               # 32 k-tiles (permuted order)
PAIRS = KT // 2             # 16 nibble pairs
CH = [(0, 512), (512, 1024), (1024, OC)]
XB = 256                    # m columns per x block
NBLK = M // XB              # 16 m-blocks
XPARTS = 4                  # per-block x load split (8 k-tiles each)
AJ_SUBS = 8                 # phase-A JIT m-subtiles (blocks 0-3)

_CACHE = {}
RUN_KWARGS = {}   # test harness can inject e.g. dict(trace=True)
LAST_RESULT = None


def _build_bass():
    import concourse.bass as bass
    import concourse.bacc as bacc
    import concourse.mybir as mybir
    from concourse.tile import TileContext

    A = mybir.AluOpType
    fp16 = mybir.dt.float16
    f32 = mybir.dt.float32
    u16 = mybir.dt.uint16

    nc = bacc.Bacc("TRN2", target_bir_lowering=False)
    xt = nc.dram_tensor("xt", [K, M], fp16, kind="ExternalInput")
    qwt = nc.dram_tensor("qwt", [K // 2, OC], u16, kind="ExternalInput")
    sbb = nc.dram_tensor("sbb", [PAIRS, 128, OC], fp16, kind="ExternalInput")
    zbb = nc.dram_tensor("zbb", [PAIRS, 128, OC], fp16, kind="ExternalInput")
    out = nc.dram_tensor("out", [M, OC], f32, kind="ExternalOutput")

    wt = nc.alloc_sbuf_tensor("wt", [128, KT * OC], fp16).ap()
    xts = [nc.alloc_sbuf_tensor(f"xtbuf{i}", [128, KT, XB], fp16).ap()
           for i in range(4)]

    xt_view = xt[:, :].rearrange("(kt p) m -> p kt m", p=128)  # [128, KT, M]
    wt3 = wt.rearrange("p (kt oc) -> p kt oc", kt=KT)

    def xpart_load(b, slot):
        for P in range(XPARTS):
            kp = KT // XPARTS
            nc.sync.dma_start(
                out=xts[slot][:, P * kp:(P + 1) * kp, :],
                in_=xt_view[:, P * kp:(P + 1) * kp, XB * b:XB * (b + 1)])

    with TileContext(nc) as tc:
        with (
            tc.tile_pool(name="deq", bufs=2) as deq,
            tc.tile_pool(name="obp", bufs=4) as obp,
            tc.tile_pool(name="acc", bufs=1, space="PSUM") as accp,
        ):
            # PE warmup: dep-free dummy matmuls start the P-state ramp
            # (and the HAM activity window on hardware) ~3us before the
            # first real matmul; results are never read and the first real
            # accumulation into this bank resets it via start=True
            dmy = deq.tile([128, 64], fp16, tag="dmy", name="dmy")
            nc.gpsimd.memset(dmy, 0.0)
            wrm = accp.tile([128, 512], f32, tag="s7", name="wrm")
            for _ in range(45):
                nc.tensor.matmul(wrm[0:64, 0:64], lhsT=dmy, rhs=dmy,
                                 start=True, stop=True)

            # pair-0 chunk-0 packed weights ride SP first: shortest DGE
            # chain, so the very first dequant starts as early as possible
            qt00 = deq.tile([128, 512], u16, tag="qt0", name="qt0")
            nc.sync.dma_start(out=qt00, in_=qwt[0:128, 0:512])

            # x blocks 0-3 for the phase-A JIT window, part-major so the
            # first k-tiles of every block land before the first matmuls
            kp = KT // XPARTS
            for P in range(XPARTS):
                for b in range(4):
                    nc.sync.dma_start(
                        out=xts[b][:, P * kp:(P + 1) * kp, :],
                        in_=xt_view[:, P * kp:(P + 1) * kp, XB * b:XB * (b + 1)])

            # ------------- dequant (two passes, slice-major) -------------
            # pass 0 covers wt columns 0:512 (all the c0-phase JIT needs)
            # for every pair first, so the PE feed is never o-column-starved;
            # pass 1 fills columns 512:OC in time for the c1/c2 phases.
            def deq_pass(t, d0, d1, sfx):
                w = d1 - d0
                if t == 0 and sfx == "0":
                    qt = qt00
                else:
                    qt = deq.tile([128, w], u16, tag="qt" + sfx,
                                  name="qt" + sfx)
                    nc.scalar.dma_start(
                        out=qt, in_=qwt[t * 128:(t + 1) * 128, d0:d1])
                sb = deq.tile([128, w], fp16, tag="sb" + sfx, name="sb" + sfx)
                nc.scalar.dma_start(out=sb, in_=sbb[t, :, d0:d1])
                zb = deq.tile([128, w], fp16, tag="zb" + sfx, name="zb" + sfx)
                nc.gpsimd.dma_start(out=zb, in_=zbb[t, :, d0:d1])

                au = deq.tile([128, w], u16, tag="au" + sfx, name="au" + sfx)
                bu = deq.tile([128, w], u16, tag="bu" + sfx, name="bu" + sfx)
                amz = deq.tile([128, w], fp16, tag="am" + sfx, name="am" + sfx)
                bmz = deq.tile([128, w], fp16, tag="bm" + sfx, name="bm" + sfx)
                nc.vector.tensor_scalar(out=au, in0=qt, scalar1=15,
                                        scalar2=0x6400, op0=A.bitwise_and,
                                        op1=A.bitwise_or)
                nc.vector.tensor_tensor(out=amz, in0=au.bitcast(fp16),
                                        in1=zb, op=A.subtract)
                nc.vector.tensor_tensor(out=wt3[:, 2 * t, d0:d1], in0=amz,
                                        in1=sb, op=A.mult)
                nc.vector.tensor_scalar(out=bu, in0=qt, scalar1=4,
                                        scalar2=0x6400,
                                        op0=A.logical_shift_right,
                                        op1=A.bitwise_or)
                seng = nc.vector if t == 0 else nc.gpsimd
                seng.tensor_tensor(out=bmz, in0=bu.bitcast(fp16),
                                   in1=zb, op=A.subtract)
                nc.vector.tensor_tensor(out=wt3[:, 2 * t + 1, d0:d1],
                                        in0=bmz, in1=sb, op=A.mult)

            for t in range(PAIRS):
                deq_pass(t, 0, 512, "0")
            for t in range(PAIRS):
                deq_pass(t, 512, OC, "1")

            # ------------- phase-A JIT GEMM (chunk-major) -------------
            def jit_mm(accs, s, kt_, c0, c1, start, stop):
                nc.tensor.matmul(
                    accs[s][:, :c1 - c0],
                    lhsT=xts[s // 2][:, kt_, (s % 2) * 128:(s % 2) * 128 + 128],
                    rhs=wt[:, kt_ * OC + c0: kt_ * OC + c1],
                    start=start, stop=stop)

            def drain(accs, s, c0, c1):
                ob = obp.tile([128, 512], f32, tag="ob", name="ob")
                nc.scalar.copy(out=ob[:, :c1 - c0], in_=accs[s][:, :c1 - c0])
                nc.gpsimd.dma_start(out=out[s * 128:(s + 1) * 128, c0:c1],
                                    in_=ob[:, :c1 - c0])

            for ci, (c0, c1) in enumerate(CH):
                accs = {s: accp.tile([128, 512], f32, tag=f"s{s}", name=f"s{s}")
                        for s in range(AJ_SUBS)}
                if ci < 2:
                    # pair-major: consume pair t right after its dequant;
                    # kt-outer so the 8 even-kt matmuls overlap the odd
                    # k-tile's dequant completing
                    for t in range(PAIRS):
                        for s in range(AJ_SUBS):
                            jit_mm(accs, s, 2 * t, c0, c1, t == 0, False)
                        for s in range(AJ_SUBS):
                            jit_mm(accs, s, 2 * t + 1, c0, c1, False,
                                   t == PAIRS - 1)
                    for s in range(AJ_SUBS):
                        drain(accs, s, c0, c1)
                else:
                    # chunk 2: subtile-major so blocks 0/1 retire early and
                    # the steady-state x prefetch overlaps the remaining JIT
                    for s in range(AJ_SUBS):
                        for kt_ in range(KT):
                            jit_mm(accs, s, kt_, c0, c1, kt_ == 0, kt_ == KT - 1)
                        drain(accs, s, c0, c1)
                        if s == 1:
                            xpart_load(4, 0)
                        elif s == 3:
                            xpart_load(5, 1)

            # ---------------- steady state: blocks 4-15 ----------------
            for b in range(4, NBLK):
                if b + 2 < NBLK:
                    xpart_load(b + 2, (b + 2) % 4)
                for sh in range(2):
                    s = 2 * b + sh
                    last = (b == NBLK - 1 and sh == 1)
                    accs = {}
                    if not last:
                        for j, (c0, c1) in enumerate(CH):
                            accs[j] = accp.tile([128, 512], f32,
                                                tag=f"s{2 * j + (s % 2)}",
                                                name=f"st{j}")
                        for kt_ in range(KT):
                            for j, (c0, c1) in enumerate(CH):
                                nc.tensor.matmul(
                                    accs[j][:, :c1 - c0],
                                    lhsT=xts[b % 4][:, kt_,
                                                    sh * 128:sh * 128 + 128],
                                    rhs=wt[:, kt_ * OC + c0: kt_ * OC + c1],
                                    start=kt_ == 0, stop=kt_ == KT - 1)
                        for j, (c0, c1) in enumerate(CH):
                            ob = obp.tile([128, 512], f32, tag="ob", name="ob")
                            nc.scalar.copy(out=ob[:, :c1 - c0],
                                           in_=accs[j][:, :c1 - c0])
                            nc.gpsimd.dma_start(
                                out=out[s * 128:(s + 1) * 128, c0:c1],
                                in_=ob[:, :c1 - c0])
                    else:
                        # last subtile: chunk-outer with a narrow final
                        # accumulation group so the end-of-kernel serial
                        # tail is one small ACT copy + one small SP DMA
                        lch = [(0, 512, 1), (512, 1024, 3),
                               (1024, 1248, 5), (1248, OC, 7)]
                        for c0, c1, tg in lch:
                            acc = accp.tile([128, 512], f32, tag=f"s{tg}",
                                            name=f"lt{tg}")
                            for kt_ in range(KT):
                                nc.tensor.matmul(
                                    acc[:, :c1 - c0],
                                    lhsT=xts[b % 4][:, kt_,
                                                    sh * 128:sh * 128 + 128],
                                    rhs=wt[:, kt_ * OC + c0: kt_ * OC + c1],
                                    start=kt_ == 0, stop=kt_ == KT - 1)
                            ob = obp.tile([128, 512], f32, tag="ob",
                                          name="ob")
                            nc.scalar.copy(out=ob[:, :c1 - c0],
                                           in_=acc[:, :c1 - c0])
                            eng = nc.sync if c1 == OC else nc.gpsimd
                            eng.dma_start(
                                out=out[s * 128:(s + 1) * 128, c0:c1],
                                in_=ob[:, :c1 - c0])

    if not nc.is_finalized():
        nc.finalize()
    return nc


def kernel(x, qweight, scales, qzeros, group_size=128, **_unused):
    global LAST_RESULT
    from concourse.bass_utils import run_bass_kernel_spmd

    if "nc" not in _CACHE:
        _CACHE["nc"] = _build_bass()
    nc = _CACHE["nc"]

    x2d = np.asarray(x).reshape(M, K)
    # k-permuted x^T: within each 256-block, even k's first then odd k's,
    # matching the nibble planes of the transposed packed weights
    xT = np.ascontiguousarray(x2d.T)                       # [K, M]
    xtp = np.ascontiguousarray(
        xT.reshape(PAIRS, 128, 2, M).transpose(0, 2, 1, 3)).reshape(K, M)

    qweight = np.asarray(qweight)
    scales = np.asarray(scales)
    qzeros = np.asarray(qzeros)

    in_maps = []
    for i in range(N_CORES):
        sl = slice(i * OC, (i + 1) * OC)
        qwc = np.ascontiguousarray(qweight[sl].astype(np.uint16).T)  # [K/2, OC]

        scc = np.asarray(scales[sl], dtype=np.float16)              # [OC, 32]
        sbb = np.empty((PAIRS, 128, OC), np.float16)
        sbb[:, 0:64, :] = scc[:, 0::2].T[:, None, :]
        sbb[:, 64:128, :] = scc[:, 1::2].T[:, None, :]

        qzc = qzeros[sl]                                            # [OC, 16]
        zlo = (1024 + (qzc & 15)).astype(np.float16)                # exact
        zhi = (1024 + ((qzc >> 4) & 15)).astype(np.float16)
        zbb = np.empty((PAIRS, 128, OC), np.float16)
        zbb[:, 0:64, :] = zlo.T[:, None, :]
        zbb[:, 64:128, :] = zhi.T[:, None, :]

        in_maps.append({
            "xt": xtp,
            "qwt": qwc,
            "sbb": np.ascontiguousarray(sbb),
            "zbb": np.ascontiguousarray(zbb),
        })

    res = run_bass_kernel_spmd(nc, in_maps, core_ids=list(range(N_CORES)),
                               **RUN_KWARGS)
    LAST_RESULT = res
    outs = [r["out"] for r in res.results]
    return np.concatenate(outs, axis=1).reshape(B, S, OUT_F).astype(np.float32)
